# revision 7
# baseline (speedup 1.0000x reference)
"""Trainium2 Bass kernel for nn_DecoderRNN (GRU decoder + InfoNCE loss).

Data-parallel over 8 NeuronCores: batch 4096 -> 512 per core; weights
replicated. Per core, per time step (T=24):
  gates  = W_hh @ h (f32r matmuls, hid on partitions) + W_ih @ xn
  r,z    = sigmoid(psum + bias)        (ACT, fused bias)
  n      = tanh((hn+b_hh_n)*r + in + b_ih_n)
  h'     = n + z*(h - n)
  logits = emb @ h'   -> exp/logsumexp + pos logit via ones-matmul reduce
  pred   = W_out @ h' + b_out  (also next step's teacher-forcing fallback)
Loss partial sums are reduced on host.
"""

import numpy as np

import concourse.bacc as bacc
import concourse.mybir as mybir
from concourse.tile import TileContext

F32 = mybir.dt.float32
F32R = mybir.dt.float32r

BS, T, H, FEAT, NEMB = 4096, 24, 1024, 13, 5
NCORES = 8
S = BS // NCORES          # batch shard per core
KT = H // 128             # hid tiles (8)
AF = mybir.ActivationFunctionType
OP = mybir.AluOpType


def build_nc(S=S, T=T, H=H):
    KT = H // 128
    nc = bacc.Bacc(None, target_bir_lowering=False)

    # ---- DRAM parameters (per-core views; names match in_maps) ----
    whhT = nc.declare_dram_parameter("whhT", [KT, 128, 3 * H], F32R, isOutput=False)
    wihT = nc.declare_dram_parameter("wihT", [FEAT, 3 * H], F32R, isOutput=False)
    embT = nc.declare_dram_parameter("embT", [128, KT, NEMB], F32R, isOutput=False)
    woutT = nc.declare_dram_parameter("woutT", [128, KT, 1], F32R, isOutput=False)
    h0T = nc.declare_dram_parameter("h0T", [KT, 128, S], F32R, isOutput=False)
    auxT = nc.declare_dram_parameter("auxT", [T, FEAT - 1, S], F32R, isOutput=False)
    ut_d = nc.declare_dram_parameter("ut", [T, 1, S], F32, isOutput=False)
    um_d = nc.declare_dram_parameter("um", [T, 1, S], F32, isOutput=False)
    negoh = nc.declare_dram_parameter("negoh", [T, NEMB, S], F32R, isOutput=False)
    brz_d = nc.declare_dram_parameter("brz", [128, 2 * KT], F32, isOutput=False)
    bhhn_d = nc.declare_dram_parameter("bhhn", [128, KT], F32, isOutput=False)
    bihn_d = nc.declare_dram_parameter("bihn", [128, KT], F32, isOutput=False)
    bout_d = nc.declare_dram_parameter("bout", [1, 1], F32, isOutput=False)
    cur0_d = nc.declare_dram_parameter("cur0", [1, S], F32, isOutput=False)

    predsT = nc.declare_dram_parameter("predsT", [T, S], F32, isOutput=True)
    loss_d = nc.declare_dram_parameter("loss", [1, 1], F32, isOutput=True)

    with TileContext(nc) as tc:
        with (
            tc.tile_pool(name="wpool", bufs=1) as wpool,
            tc.tile_pool(name="hpool", bufs=2) as hpool,
            tc.tile_pool(name="gpool", bufs=2) as gpool,
            tc.tile_pool(name="spool", bufs=2) as spool,
            tc.tile_pool(name="iopool", bufs=2) as iopool,
            tc.tile_pool(name="accp", bufs=1) as accp,
            tc.tile_pool(name="ppr", bufs=2, space="PSUM") as ppr,
            tc.tile_pool(name="ppz", bufs=2, space="PSUM") as ppz,
            tc.tile_pool(name="pphn", bufs=1, space="PSUM") as pphn,
            tc.tile_pool(name="ppin", bufs=1, space="PSUM") as ppin,
            tc.tile_pool(name="ppm", bufs=1, space="PSUM") as ppm,
            tc.tile_pool(name="ppp", bufs=1, space="PSUM") as ppp,
        ):
            # ---- load weights / constants (once) ----
            whh_sb = wpool.tile([128, KT, 3 * H], F32R, tag="whh")
            for k in range(KT):
                nc.sync.dma_start(out=whh_sb[:, k, :], in_=whhT[k])
            wih_sb = wpool.tile([FEAT, 3 * H], F32R, tag="wih")
            nc.sync.dma_start(out=wih_sb, in_=wihT[:, :])
            emb_sb = wpool.tile([128, KT, NEMB], F32R, tag="emb")
            nc.sync.dma_start(out=emb_sb, in_=embT[:, :, :])
            wout_sb = wpool.tile([128, KT, 1], F32R, tag="wout")
            nc.sync.dma_start(out=wout_sb, in_=woutT[:, :, :])
            brz_sb = wpool.tile([128, 2 * KT], F32, tag="brz")
            nc.sync.dma_start(out=brz_sb, in_=brz_d[:, :])
            bhhn_sb = wpool.tile([128, KT], F32, tag="bhhn")
            nc.sync.dma_start(out=bhhn_sb, in_=bhhn_d[:, :])
            bihn_sb = wpool.tile([128, KT], F32, tag="bihn")
            nc.sync.dma_start(out=bihn_sb, in_=bihn_d[:, :])
            bout_sb = wpool.tile([1, 1], F32, tag="bout")
            nc.sync.dma_start(out=bout_sb, in_=bout_d[:, :])
            ones_f32 = wpool.tile([NEMB, 1], F32, tag="ones_f32")
            nc.vector.memset(ones_f32, 1.0)
            ones_sb = wpool.tile([NEMB, 1], F32R, tag="ones")
            nc.vector.tensor_copy(out=ones_sb, in_=ones_f32)

            acc = accp.tile([1, S], F32, tag="acc")
            nc.vector.memset(acc, 0.0)

            # ---- initial state ----
            h_prev = []
            for k in range(KT):
                ht = hpool.tile([128, S], F32R, tag=f"h{k}")
                nc.sync.dma_start(out=ht, in_=h0T[k])
                h_prev.append(ht)
            cur = spool.tile([1, S], F32, tag="cur")
            nc.sync.dma_start(out=cur, in_=cur0_d[:, :])

            # deferred loss-tail state from the previous step
            pend = {}

            def emit_red(t):
                ps_red = ppm.tile([1, S], F32, tag="misc")
                nc.tensor.matmul(ps_red, ones_sb, pend["expl"], start=True, stop=True)
                lossb = spool.tile([1, S], F32, tag="lossb")
                nc.scalar.activation(out=lossb, in_=ps_red, func=AF.Ln)
                pend["lossb"] = lossb

            def emit_pos(t):
                ps_pos = ppm.tile([1, S], F32, tag="misc")
                nc.tensor.matmul(ps_pos, ones_sb, pend["poslg"], start=True, stop=True)
                losstb = spool.tile([1, S], F32, tag="losstb")
                nc.vector.tensor_add(losstb, pend["lossb"], ps_pos)
                nc.vector.tensor_add(acc, acc, losstb)

            for t in range(T):
                # ---- per-step inputs + decoder input (teacher forcing) ----
                xn = iopool.tile([FEAT, S], F32R, tag="xn")
                nc.sync.dma_start(out=xn[1:FEAT, :], in_=auxT[t])
                utt = iopool.tile([1, S], F32, tag="ut")
                nc.sync.dma_start(out=utt, in_=ut_d[t])
                umt = iopool.tile([1, S], F32, tag="um")
                nc.sync.dma_start(out=umt, in_=um_d[t])
                oht = iopool.tile([NEMB, S], F32R, tag="oh")
                nc.sync.dma_start(out=oht, in_=negoh[t])

                tmpd = spool.tile([1, S], F32, tag="tmpd")
                nc.vector.tensor_mul(tmpd, cur, umt)
                nc.vector.tensor_add(xn[0:1, :], tmpd, utt)

                h_new = [hpool.tile([128, S], F32R, tag=f"h{k}", name=f"h{k}")
                         for k in range(KT)]

                for jj in range(KT):
                    # r gate
                    ps_r = ppr.tile([128, S], F32, tag="r")
                    c = jj * 128
                    for k in range(KT):
                        nc.tensor.matmul(ps_r, whh_sb[:, k, c:c + 128], h_prev[k],
                                         start=(k == 0), stop=False)
                    nc.tensor.matmul(ps_r, wih_sb[:, c:c + 128], xn,
                                     start=False, stop=True)
                    # z gate
                    ps_z = ppz.tile([128, S], F32, tag="z")
                    c = H + jj * 128
                    for k in range(KT):
                        nc.tensor.matmul(ps_z, whh_sb[:, k, c:c + 128], h_prev[k],
                                         start=(k == 0), stop=False)
                    nc.tensor.matmul(ps_z, wih_sb[:, c:c + 128], xn,
                                     start=False, stop=True)
                    # n gate: hn & in kept separate
                    ps_hn = pphn.tile([128, S], F32, tag="hn")
                    c = 2 * H + jj * 128
                    for k in range(KT):
                        nc.tensor.matmul(ps_hn, whh_sb[:, k, c:c + 128], h_prev[k],
                                         start=(k == 0), stop=(k == KT - 1))
                    ps_in = ppin.tile([128, S], F32, tag="in")
                    nc.tensor.matmul(ps_in, wih_sb[:, c:c + 128], xn,
                                     start=True, stop=True)

                    # interleave previous step's loss-tail reduce matmuls here
                    if jj == 0 and pend.get("expl") is not None:
                        emit_red(t - 1)
                    if jj == 1 and pend.get("poslg") is not None:
                        emit_pos(t - 1)
                        pend.clear()

                    r_sb = gpool.tile([128, S], F32, tag="r_sb")
                    nc.scalar.activation(out=r_sb, in_=ps_r, func=AF.Sigmoid,
                                         bias=brz_sb[:, jj:jj + 1], scale=1.0)
                    z_sb = gpool.tile([128, S], F32, tag="z_sb")
                    nc.scalar.activation(out=z_sb, in_=ps_z, func=AF.Sigmoid,
                                         bias=brz_sb[:, KT + jj:KT + jj + 1], scale=1.0)
                    t2 = gpool.tile([128, S], F32, tag="t2")
                    nc.vector.scalar_tensor_tensor(t2, ps_hn, bhhn_sb[:, jj:jj + 1],
                                                   r_sb, OP.add, OP.mult)
                    nc.vector.tensor_add(t2, t2, ps_in)
                    n_sb = gpool.tile([128, S], F32, tag="n_sb")
                    nc.scalar.activation(out=n_sb, in_=t2, func=AF.Tanh,
                                         bias=bihn_sb[:, jj:jj + 1], scale=1.0)
                    nc.vector.tensor_sub(h_new[jj], h_prev[jj], n_sb)
                    nc.vector.tensor_mul(h_new[jj], z_sb, h_new[jj])
                    nc.vector.tensor_add(h_new[jj], h_new[jj], n_sb)

                # ---- step boundary: logits, preds ----
                ps_lg = ppm.tile([NEMB, S], F32, tag="misc")
                for k in range(KT):
                    nc.tensor.matmul(ps_lg, emb_sb[:, k, :], h_new[k],
                                     start=(k == 0), stop=(k == KT - 1))
                ps_pr = ppp.tile([1, S], F32, tag="pred")
                for k in range(KT):
                    nc.tensor.matmul(ps_pr, wout_sb[:, k, :], h_new[k],
                                     start=(k == 0), stop=(k == KT - 1))

                expl = spool.tile([NEMB, S], F32R, tag="expl")
                nc.scalar.activation(out=expl, in_=ps_lg, func=AF.Exp)
                poslg = spool.tile([NEMB, S], F32R, tag="poslg")
                nc.vector.tensor_mul(poslg, ps_lg, oht)
                pend["expl"] = expl
                pend["poslg"] = poslg

                cur = spool.tile([1, S], F32, tag="cur")
                nc.vector.tensor_scalar_add(cur, ps_pr, bout_sb[0:1, 0:1])
                nc.sync.dma_start(out=predsT[t:t + 1, :], in_=cur)

                h_prev = h_new

            # final loss tail for the last step
            emit_red(T - 1)
            emit_pos(T - 1)
            lsum = spool.tile([1, 1], F32, tag="lsum")
            nc.vector.tensor_reduce(lsum, acc, axis=mybir.AxisListType.X, op=OP.add)
            nc.sync.dma_start(out=loss_d[:, :], in_=lsum)

    return nc


def prep_core_inputs(inputs, n_cores=NCORES):
    """Host-side shard prep. Returns list of in_maps (one per core)."""
    aux = np.asarray(inputs["auxiliary"], np.float32)          # [BS,T,12]
    cur0 = np.asarray(inputs["current_pm25"], np.float32)      # [BS,1]
    h0 = np.asarray(inputs["hn"], np.float32)[0]               # [BS,H]
    st = np.asarray(inputs["st_level"])                        # [BS,T,1] int
    tgt_seq = np.asarray(inputs["tgt_seq"], np.float32)        # [BS,T,1]
    v = np.asarray(inputs["v"], np.float32)                    # [BS,T]
    W_ih = np.asarray(inputs["W_ih"], np.float32)              # [3H,FEAT]
    W_hh = np.asarray(inputs["W_hh"], np.float32)              # [3H,H]
    b_ih = np.asarray(inputs["b_ih"], np.float32)              # [3H]
    b_hh = np.asarray(inputs["b_hh"], np.float32)              # [3H]
    emb = np.asarray(inputs["emb"], np.float32)                # [NEMB,H]
    W_out = np.asarray(inputs["W_out"], np.float32)            # [1,H]
    b_out = np.asarray(inputs["b_out"], np.float32)            # [1]

    bs, t_, _ = aux.shape
    h = W_hh.shape[1]
    kt = h // 128
    s = bs // n_cores

    # teacher forcing select coefficients
    tgtf = np.concatenate([cur0[:, None, :], tgt_seq[:, :-1, :1]], axis=1)  # [BS,T,1]
    mask = np.where(tgtf != 0, v[:, :, None], np.zeros_like(v[:, :, None]))
    use = (mask == 1.0).astype(np.float32)
    use[:, 0, :] = 0.0                       # t==0 always uses cur
    ut = use * tgtf                          # [BS,T,1]
    um = 1.0 - use

    onehot = np.eye(NEMB, dtype=np.float32)[st[:, :, 0]]       # [BS,T,NEMB]
    noh = -onehot

    shared = {
        "whhT": np.ascontiguousarray(W_hh.T.reshape(kt, 128, 3 * h)),
        "wihT": np.ascontiguousarray(W_ih.T),
        "embT": np.ascontiguousarray(emb.T.reshape(kt, 128, NEMB).transpose(1, 0, 2)),
        "woutT": np.ascontiguousarray(W_out.T.reshape(kt, 128, 1).transpose(1, 0, 2)),
        "brz": np.ascontiguousarray((b_ih + b_hh)[:2 * h].reshape(2 * kt, 128).T),
        "bhhn": np.ascontiguousarray(b_hh[2 * h:].reshape(kt, 128).T),
        "bihn": np.ascontiguousarray(b_ih[2 * h:].reshape(kt, 128).T),
        "bout": b_out.reshape(1, 1).astype(np.float32),
    }
    maps = []
    for c in range(n_cores):
        r0, r1 = c * s, (c + 1) * s
        m = dict(shared)
        m["auxT"] = np.ascontiguousarray(aux[r0:r1].transpose(1, 2, 0))        # [T,12,S]
        m["ut"] = np.ascontiguousarray(ut[r0:r1, :, 0].T.reshape(t_, 1, s))
        m["um"] = np.ascontiguousarray(um[r0:r1, :, 0].T.reshape(t_, 1, s))
        m["negoh"] = np.ascontiguousarray(noh[r0:r1].transpose(1, 2, 0))       # [T,5,S]
        m["h0T"] = np.ascontiguousarray(h0[r0:r1].T.reshape(kt, 128, s))
        m["cur0"] = np.ascontiguousarray(cur0[r0:r1].T)                        # [1,S]
        maps.append(m)
    return maps


_NC = None


def _get_nc():
    global _NC
    if _NC is None:
        nc = build_nc()
        nc.finalize()
        _NC = nc
    return _NC


def run_cores(inputs, trace=False):
    from concourse.bass_utils import run_bass_kernel_spmd
    nc = _get_nc()
    maps = prep_core_inputs(inputs)
    res = run_bass_kernel_spmd(nc, maps, core_ids=list(range(NCORES)),
                               trace=trace)
    return res


def assemble(results):
    preds = np.empty((BS, T, 1), np.float32)
    loss_sum = 0.0
    for c, out in enumerate(results):
        preds[c * S:(c + 1) * S, :, 0] = out["predsT"].T
        loss_sum += float(out["loss"][0, 0])
    loss = np.float32(loss_sum / (BS * T))
    return preds, loss


def kernel(**inputs):
    res = run_cores(inputs, trace=False)
    return assemble(res.results)
